# revision 37
# baseline (speedup 1.0000x reference)
"""Trainium2 Bass kernel for dilated sliding-window attention (AttnWrapper).

Reference computation (all fp32):
  combined = [begin | main | end]                       # [8256, 768]
  keys[t]  = combined[t + 32 + off], off in +-{4..32}   # 16 dilated window keys
  q = (main @ wq.T + bq) * 96**-0.5
  k/v = keys @ w{k,v}.T + b{k,v}
  attn = softmax(q.k), ctx = attn.v, out = [main | ctx @ wo.T + bo]

Sharding: tokens across 8 cores (1024 each) with a 64-row halo of the
combined buffer; weights replicated. Each core computes attn_outT
[768, 1024] in bf16; the host transposes and concatenates with main.

Measured hardware facts this schedule is built around:
 - the PE sustains 1.2 GHz (DR fp8 512-col matmul = 216 ns, never less),
   so PE cycles are the budget: ~60 us of matmul work; every extra PE op
   (e.g. broadcast matmuls) is a real cost;
 - each DMA queue sustains only ~50 GB/s regardless of transfer size or
   queue (HWDGE/SWDGE alike); three queues in parallel ~150 GB/s, so the
   3.4 MB of inputs need ~8 us spread across all three queues with the
   first-needed tensors (x, wv) split between queues;
 - Pool (Q7) compute is slow (~900 ns for a [128,384] TT, ~2 us for a
   [96,1024] partition broadcast) but runs concurrently; DVE is the
   attention-phase wall, so work is split DVE/Pool deliberately.

Device-side structure:
 - All projections are fp8 DoubleRow matmuls; weights pre-scaled by 2^6
   on host (q also carries 96^-1/2). q/k drains keep the scale (exp
   absorbs 2^-12); v drains are scale-free casts (the normalize STT
   multiplies by 2^-6); out drains scale by 2^-6. Host pre-arranges all
   weights into SBUF layout (wq/wk head-major) so per-head 74 KB slices
   stream in just ahead of their projection.
 - Scores are S[key, token] per 512-token group: 5 chunks of 128 keys
   packed into two [128,384] PSUM tiles -> 2 exps + 2 mask multiplies
   per head (masks host-built; the 64-key tail chunk is extended to 128
   rows, kTh/vt zero-padded, mask zeroes the overhang).
 - A ones-column per V head block makes the ctx matmul emit the softmax
   denominator as PSUM row 96. Per pair: two DVE reciprocals off the
   PSUM rows into one [1,1024] tile, one Pool partition_broadcast to
   [96,1024] f32, then per head one DVE scalar_tensor_tensor fusing
   (ctx * 2^-6) * (1/den) + fp8 cast, emitted under the next pair's
   scores so the in-order PE queue never waits on the DVE/Pool chain.
 - Emission is g-major; group 0's out-projection chunks + output DMAs
   are interleaved into group 1's attention; drains alternate ACT/DVE.
"""

import numpy as np

EMBED_DIM = 768
NUM_HEADS = 8
HEAD_DIM = 96
OVERLAP = 32
HALO = 2 * OVERLAP          # 64 extra combined rows per core
N_LINES = 8192
N_CORES = 8
TOK = N_LINES // N_CORES    # 1024 tokens per core
ROWS = TOK + HALO           # 1088 combined rows per core
KPAD = ROWS + 64            # kTh padded so the tail key chunk is 128 wide
GRP = 512                   # tokens per attention group
NG = TOK // GRP             # 2 groups
VBLK = HEAD_DIM + 1         # 97: v head block + ones column
KC = EMBED_DIM // 128       # 6 contraction chunks of 128
DC = KC // 2                # 3 DoubleRow chunks of 256
NVC = (ROWS + 127) // 128   # 9 v row-chunks (8x128 + 64)
WS = 64.0                   # 2^6 fp8 weight pre-scale
HB = KC * HEAD_DIM          # 576 cols per head in head-major w layout

# score chunks: (k0, w0, w1, tile, col) -- key rows [k0,k0+128) vs token
# window [w0,w1), packed at `col` of s-tile A or B (each [128, 384])
SCHUNKS = [
    (0,   0,   128, 0, 0),
    (128, 64,  256, 0, 128),
    (512, 448, 512, 0, 320),
    (256, 192, 384, 1, 0),
    (384, 320, 512, 1, 192),
]


def _build_program():
    import concourse.bacc as bacc
    import concourse.mybir as mybir
    from concourse.tile import TileContext

    f32 = mybir.dt.float32
    bf16 = mybir.dt.bfloat16
    f8 = mybir.dt.float8e4
    DR = mybir.MatmulPerfMode.DoubleRow
    ACT = mybir.ActivationFunctionType
    ALU = mybir.AluOpType
    D = EMBED_DIM

    nc = bacc.Bacc("TRN2", target_bir_lowering=False, debug=False,
                   enable_asserts=False, num_devices=N_CORES)

    x8 = nc.dram_tensor("x8", [128, KC * ROWS], f8, kind="ExternalInput")
    wq8 = nc.dram_tensor("wq8", [128, KC * D], f8, kind="ExternalInput")
    wk8 = nc.dram_tensor("wk8", [128, KC * D], f8, kind="ExternalInput")
    wv8 = nc.dram_tensor("wv8", [128, KC * D], f8, kind="ExternalInput")
    wo8 = nc.dram_tensor("wo8", [HEAD_DIM, NUM_HEADS * D], f8,
                         kind="ExternalInput")
    bq = nc.dram_tensor("bq", [HEAD_DIM, NUM_HEADS], f32, kind="ExternalInput")
    bo2 = nc.dram_tensor("bo2", [128, KC], f32, kind="ExternalInput")
    mAd = nc.dram_tensor("mA", [128, 384], bf16, kind="ExternalInput")
    mBd = nc.dram_tensor("mB", [128, 384], bf16, kind="ExternalInput")
    out = nc.dram_tensor("out", [D, TOK], bf16, kind="ExternalOutput")

    def pair(tile, dc, lo, sz, inner):
        # [128, 2, sz] DoubleRow view of contraction chunk pair dc
        return tile[:, 2 * dc * inner:(2 * dc + 2) * inner] \
            .rearrange("p (i n) -> p i n", i=2)[:, :, lo:lo + sz]

    with TileContext(nc) as tc:
        with tc.tile_pool(name="persist", bufs=1) as pers:
            qTh = [pers.tile([HEAD_DIM, TOK], bf16, name=f"qTh{h}")
                   for h in range(NUM_HEADS)]
            kTh = [pers.tile([HEAD_DIM, KPAD], bf16, name=f"kTh{h}")
                   for h in range(NUM_HEADS)]
            vt = [pers.tile([128, NUM_HEADS * VBLK], bf16, name=f"vt{r}")
                  for r in range(NVC)]
            ctxH = [pers.tile([HEAD_DIM, NUM_HEADS * GRP], f8, name=f"ctxH{g}")
                    for g in range(NG)]
            bqt = pers.tile([HEAD_DIM, NUM_HEADS], f32)
            bo2t = pers.tile([128, KC], f32)
            mA = pers.tile([128, 384], bf16)
            mB = pers.tile([128, 384], bf16)
            ones = pers.tile([1, HEAD_DIM], bf16)
            xt = pers.tile([128, KC * ROWS], f8, name="xt")
            wvt = pers.tile([128, KC * D], f8, name="w_v")
            wqt = pers.tile([128, KC * D], f8, name="w_q")
            wkt = pers.tile([128, KC * D], f8, name="w_k")
            wot = pers.tile([HEAD_DIM, NUM_HEADS * D], f8, name="w_o")

            for r in range(NVC):
                rows = min(128, ROWS - 128 * r)
                dst = vt[r][0:rows, :].rearrange("p (b c) -> p b c", c=VBLK)
                nc.gpsimd.memset(dst[:, :, 0:1], 1.0)
            # zero the pad regions read by the extended tail key chunk
            nc.gpsimd.memset(vt[NVC - 1][64:128, :], 0.0)
            for h in range(NUM_HEADS):
                nc.gpsimd.memset(kTh[h][:, ROWS:KPAD], 0.0)

            # one large DMA per tensor in need order (v-proj first);
            # host pre-arranged everything so transfers are flat
            nc.sync.dma_start(xt[:], x8.ap())
            nc.scalar.dma_start(wvt[:], wv8.ap())
            nc.gpsimd.dma_start(mA[:], mAd.ap())
            nc.gpsimd.dma_start(mB[:], mBd.ap())
            nc.gpsimd.dma_start(bqt[:], bq.ap())
            nc.gpsimd.dma_start(bo2t[:], bo2.ap())
            nc.sync.dma_start(wkt[:], wk8.ap())
            nc.scalar.dma_start(wqt[:], wq8.ap())
            # wo is deferred into the attention phase (see below)

            with tc.tile_pool(name="vpsum", bufs=2, space="PSUM") as vpsum:
                # ---- v projection (x-stationary, fp8 DoubleRow)
                for r in range(NVC):
                    rows = min(128, ROWS - 128 * r)
                    pv0 = vpsum.tile([128, 512], f32, tag="pv0", name="pv0")
                    pv1 = vpsum.tile([128, 256], f32, tag="pv1", name="pv1")
                    for dc in range(DC):
                        lhs = pair(xt, dc, 128 * r, rows, ROWS)
                        nc.tensor.matmul(pv0[0:rows, :], lhs,
                                         pair(wvt, dc, 0, 512, D),
                                         start=(dc == 0), stop=(dc == DC - 1),
                                         perf_mode=DR)
                        nc.tensor.matmul(pv1[0:rows, :], lhs,
                                         pair(wvt, dc, 512, 256, D),
                                         start=(dc == 0), stop=(dc == DC - 1),
                                         perf_mode=DR)
                    # scale-free drains (1/WS folded into the normalize STT)
                    dst = vt[r][0:rows, :].rearrange("p (b c) -> p b c",
                                                     c=VBLK)
                    big_src = pv0[0:rows, 0:5 * HEAD_DIM] \
                        .rearrange("p (b c) -> p b c", c=HEAD_DIM)
                    if r % 2 == 0:
                        nc.scalar.activation(dst[:, 0:5, 0:HEAD_DIM], big_src,
                                             ACT.Copy, scale=1.0)
                    else:
                        nc.vector.tensor_copy(dst[:, 0:5, 0:HEAD_DIM],
                                              big_src)
                    nc.vector.tensor_copy(dst[:, 5, 0:32],
                                          pv0[0:rows, 480:512])
                    nc.vector.tensor_copy(dst[:, 5, 32:HEAD_DIM],
                                          pv1[0:rows, 0:64])
                    nc.vector.tensor_copy(
                        dst[:, 6:8, 0:HEAD_DIM],
                        pv1[0:rows, 64:64 + 2 * HEAD_DIM]
                        .rearrange("p (b c) -> p b c", c=HEAD_DIM))

            with tc.tile_pool(name="ppsum", bufs=4, space="PSUM") as ppsum:
                # ---- q / k projections (weight-stationary, fp8 DoubleRow)
                for h in range(NUM_HEADS):
                    for n0 in (0, 512):
                        ps = ppsum.tile([HEAD_DIM, 512], f32, tag="pqk",
                                        name="ps_q")
                        for dc in range(DC):
                            nc.tensor.matmul(
                                ps[:],
                                pair(wqt, dc, h * HEAD_DIM, HEAD_DIM, D),
                                pair(xt, dc, OVERLAP + n0, 512, ROWS),
                                start=(dc == 0), stop=(dc == DC - 1),
                                perf_mode=DR)
                        # qTh keeps the 2^6 weight scale; exp absorbs it
                        nc.scalar.activation(qTh[h][:, n0:n0 + 512], ps[:],
                                             ACT.Identity,
                                             bias=bqt[:, h:h + 1], scale=1.0)
                    for n0, sz in ((0, 512), (512, 512), (1024, 64)):
                        ps = ppsum.tile([HEAD_DIM, 512], f32, tag="pqk",
                                        name="ps_k")
                        for dc in range(DC):
                            nc.tensor.matmul(
                                ps[:, 0:sz],
                                pair(wkt, dc, h * HEAD_DIM, HEAD_DIM, D),
                                pair(xt, dc, n0, sz, ROWS),
                                start=(dc == 0), stop=(dc == DC - 1),
                                perf_mode=DR)
                        nc.vector.tensor_copy(kTh[h][:, n0:n0 + sz],
                                              ps[:, 0:sz])

            # ---- attention + normalization + out-projection, pipelined
            with tc.tile_pool(name="apool", bufs=2) as apool, \
                 tc.tile_pool(name="opool", bufs=6) as opool, \
                 tc.tile_pool(name="apsum", bufs=2, space="PSUM") as apsum:

                dmaq = [nc.sync, nc.scalar, nc.gpsimd]
                state = {"q": 0}

                def scores(g, h):
                    base = GRP * g
                    st = [apsum.tile([128, 384], f32, tag="s", name="s_ps",
                                     bufs=3) for _ in range(2)]
                    for k0, w0, w1, ti, col in SCHUNKS:
                        nc.tensor.matmul(
                            st[ti][:, col:col + (w1 - w0)],
                            kTh[h][:, base + k0:base + k0 + 128],
                            qTh[h][:, base + w0:base + w1],
                            start=True, stop=True, skip_group_check=True)
                    return st

                def exp_mask_ctx(g, h, st):
                    ex = [apool.tile([128, 384], bf16, tag="ex", name="ex",
                                     bufs=6) for _ in range(2)]
                    for ti in range(2):
                        # scores carry 2^12 from the two 2^6 weight scales
                        nc.scalar.activation(ex[ti][:], st[ti][:],
                                             ACT.Exp, scale=1.0 / (WS * WS))
                        # both masks on Pool: its queue is short, so ex is
                        # ready sooner than behind DVE's backlog
                        nc.gpsimd.tensor_tensor(
                            out=ex[ti][:], in0=ex[ti][:],
                            in1=(mA if ti == 0 else mB)[:], op=ALU.mult)
                    ctx = apsum.tile([VBLK, GRP], f32, tag="ctx",
                                     name="ctx_ps", bufs=3)
                    vi = 4 * g
                    for i, (k0, w0, w1, ti, col) in enumerate(SCHUNKS):
                        nc.tensor.matmul(
                            ctx[:, w0:w1],
                            vt[vi + k0 // 128][0:128,
                                               h * VBLK:(h + 1) * VBLK],
                            ex[ti][:, col:col + (w1 - w0)],
                            start=(i == 0), stop=(i == len(SCHUNKS) - 1),
                            skip_group_check=True)
                    return ctx

                def den_cast(ctx, use_act):
                    # stage the PSUM denominator row as bf16 for the PE bcast
                    rrb = apool.tile([1, GRP], bf16, tag="rrb", name="rrb",
                                     bufs=3)
                    if use_act:
                        nc.scalar.activation(rrb[:], ctx[HEAD_DIM:VBLK, :],
                                             ACT.Copy, scale=1.0)
                    else:
                        nc.vector.tensor_copy(rrb[:], ctx[HEAD_DIM:VBLK, :])
                    return rrb

                def bcast_stt(g, h, ctx, rrb):
                    # PE broadcasts den down 96 partitions; DVE reciprocal
                    # moves it to SBUF; STT fuses (ctx/2^6)*(1/den) + fp8 cast
                    bc = apsum.tile([128, GRP], f32, tag="big", name="bc",
                                    bufs=2)
                    nc.tensor.matmul(bc[0:HEAD_DIM, :], ones[:], rrb[:],
                                     start=True, stop=True)
                    rrB = apool.tile([HEAD_DIM, GRP], f32, tag="rrB",
                                     name="rrB", bufs=2)
                    nc.vector.reciprocal_approx_fast(rrB[:],
                                                     bc[0:HEAD_DIM, :])
                    nc.vector.scalar_tensor_tensor(
                        out=ctxH[g][:, h * GRP:(h + 1) * GRP],
                        in0=ctx[0:HEAD_DIM, :], scalar=1.0 / WS,
                        in1=rrB[:], op0=ALU.mult, op1=ALU.mult)

                def outproj_chunk(g, dc, last=False):
                    op = apsum.tile([128, GRP], f32, tag="big", name="ps_o",
                                    bufs=2)
                    for hp in range(NUM_HEADS // 2):
                        nc.tensor.matmul(
                            op[:],
                            pair(wot, hp, dc * 128, 128, D),
                            ctxH[g][:, 2 * hp * GRP:(2 * hp + 2) * GRP]
                            .rearrange("p (i n) -> p i n", i=2),
                            start=(hp == 0), stop=(hp == NUM_HEADS // 2 - 1),
                            perf_mode=DR)
                    ost = opool.tile([128, GRP], bf16, tag="ost", name="ost")
                    if dc % 2 == 0:
                        nc.scalar.activation(ost[:], op[:], ACT.Identity,
                                             bias=bo2t[:, dc:dc + 1],
                                             scale=1.0 / WS)
                    else:
                        nc.vector.tensor_scalar(
                            out=ost[:], in0=op[:], scalar1=1.0 / WS,
                            scalar2=bo2t[:, dc:dc + 1],
                            op0=ALU.mult, op1=ALU.add)
                    dst = out.ap()[dc * 128:(dc + 1) * 128,
                                   g * GRP:(g + 1) * GRP]
                    if last:
                        # halve the tail exposure: two queues per chunk
                        for i in range(2):
                            dmaq[state["q"] % 3].dma_start(
                                dst[:, i * 256:(i + 1) * 256],
                                ost[:, i * 256:(i + 1) * 256])
                            state["q"] += 1
                    else:
                        dmaq[state["q"] % 3].dma_start(dst, ost[:])
                        state["q"] += 1

                pending = None
                # outproj chunks of g=0 interleaved after g=1 pairs
                op_sched = {4: [0], 5: [1, 2], 6: [3, 4]}
                pi = 0
                for g in range(NG):
                    for hp in range(NUM_HEADS // 2):
                        h0, h1 = 2 * hp, 2 * hp + 1
                        st0 = scores(g, h0)
                        st1 = scores(g, h1)
                        if pending is not None:
                            bcast_stt(*pending)
                            pending = None
                        ctx0 = exp_mask_ctx(g, h0, st0)
                        rrb0 = den_cast(ctx0, use_act=False)
                        ctx1 = exp_mask_ctx(g, h1, st1)
                        bcast_stt(g, h0, ctx0, rrb0)
                        rrb1 = den_cast(ctx1, use_act=False)
                        pending = (g, h1, ctx1, rrb1)
                        if pi == 0:
                            # wo lands well before outproj(g=0) needs it
                            nc.gpsimd.dma_start(wot[:], wo8.ap())
                        for dc in op_sched.get(pi, []):
                            outproj_chunk(0, dc)
                        pi += 1
                if pending is not None:
                    bcast_stt(*pending)
                # last g0 chunk here covers the final normalize chain
                outproj_chunk(0, 5)
                for dc in range(KC):
                    outproj_chunk(1, dc, last=(dc >= 4))
    nc.compile()
    return nc


_program_cache = {}


def _get_program():
    if "nc" not in _program_cache:
        _program_cache["nc"] = _build_program()
    return _program_cache["nc"]


def _host_masks():
    # band validity: combined-key - token = d, d in [0,64], d%4==0, d!=32
    import ml_dtypes

    def band(nk, nt, off):
        kk, tt = np.meshgrid(np.arange(nk), np.arange(nt), indexing="ij")
        d = kk - tt + off
        valid = (d >= 0) & (d <= HALO) & (d % 4 == 0) & (d != OVERLAP)
        return valid.astype(ml_dtypes.bfloat16)

    mA = np.concatenate([band(128, 128, 0), band(128, 192, HALO),
                         band(128, 64, HALO)], axis=1)
    mB = np.concatenate([band(128, 192, HALO), band(128, 192, HALO)], axis=1)
    return np.ascontiguousarray(mA), np.ascontiguousarray(mB)


def kernel(main, begin, end, in_proj_w, in_proj_b, out_proj_w, out_proj_b):
    import ml_dtypes
    from concourse.bass_utils import run_bass_kernel_spmd

    f8np = ml_dtypes.float8_e4m3

    main = np.asarray(main, np.float32)
    begin = np.asarray(begin, np.float32)
    end = np.asarray(end, np.float32)
    in_proj_w = np.asarray(in_proj_w, np.float32)
    in_proj_b = np.asarray(in_proj_b, np.float32)
    out_proj_w = np.asarray(out_proj_w, np.float32)
    out_proj_b = np.asarray(out_proj_b, np.float32)

    D = EMBED_DIM
    scale = HEAD_DIM ** -0.5
    wq, wk, wv = in_proj_w[:D], in_proj_w[D:2 * D], in_proj_w[2 * D:]
    bq_, bv = in_proj_b[:D], in_proj_b[2 * D:3 * D]
    combined = np.concatenate([begin, main, end], axis=0)  # [N + 64, D]

    def chunk_major(a, p):
        # [C*p, N] -> [p, C*N]: row c*p+q lands at partition q, block c
        c = a.shape[0] // p
        return np.ascontiguousarray(
            a.reshape(c, p, -1).transpose(1, 0, 2).reshape(p, -1))

    wq8 = chunk_major(wq.T * (scale * WS), 128).astype(f8np)
    wk8 = chunk_major(wk.T * WS, 128).astype(f8np)
    wv8 = chunk_major(wv.T * WS, 128).astype(f8np)
    wo8 = chunk_major(out_proj_w.T * WS, HEAD_DIM).astype(f8np)
    bq_heads = np.ascontiguousarray(
        (bq_ * (scale * WS)).reshape(NUM_HEADS, HEAD_DIM).T)
    bo2 = out_proj_w @ bv + out_proj_b                      # [768]
    bo2_chunks = np.ascontiguousarray(bo2.reshape(KC, 128).T)
    mA, mB = _host_masks()

    shared = {
        "wq8": wq8, "wk8": wk8, "wv8": wv8, "wo8": wo8,
        "bq": bq_heads, "bo2": bo2_chunks, "mA": mA, "mB": mB,
    }
    in_maps = []
    for c in range(N_CORES):
        x8c = chunk_major(
            combined[c * TOK: c * TOK + ROWS].T, 128).astype(f8np)
        in_maps.append({**shared, "x8": x8c})

    nc = _get_program()
    res = run_bass_kernel_spmd(nc, in_maps, core_ids=list(range(N_CORES)),
                               **_program_cache.get("run_kwargs", {}))
    _program_cache["last_result"] = res

    outp = np.empty((N_LINES, 2 * D), np.float32)
    outp[:, :D] = main
    for c in range(N_CORES):
        outp[c * TOK:(c + 1) * TOK, D:] = \
            res.results[c]["out"].astype(np.float32).T
    return outp
